# revision 2
# baseline (speedup 1.0000x reference)
"""Bass/Trainium2 kernel for nn_GPREDecoder (GlobalPointer relation-extraction loss).

Strategy: data-parallel over batch (B=8 -> 8 cores, 1 example per core).
Per example the device computes:
  - projT = W_all @ x_aug.T in fp8(e4m3) inputs -> psum f32, streamed so the
    PE consumes input kt-chunks as the DMA delivers them
  - the projection output channel order is PERMUTED so the four non-rope
    groups (q_head, k_head, q_tail, k_tail) land row-aligned (rows 0..67) in
    proj tiles 0..3 -> the head/tail exp streams start right after evac
  - the four rope (ent) groups are spilled/assembled in the background, then
    RoPE-rotated (J-matmul + cos/sin elementwise in bf16)
  - per-head S x S logit tiles on PE (bf16), exp(SCALE*logit) on ACT with
    fused per-row accumulation -> per-head sum(exp(logits))
  - tril heads: sub-diagonal blocks are simply never computed; the DIAGONAL
    128x128 blocks are left off-device entirely (host adds their upper-tri
    exp-sum from the returned q/k), so no masks exist on the device
The host gathers the 64 ground-truth pairs per head from q/k, computes the
diagonal-block corrections for the tril heads, applies the multilabel-CE
pos/neg log corrections in float64, and returns the scalar loss.
"""

import ml_dtypes
import numpy as np
from contextlib import ExitStack

import concourse.bass as bass
import concourse.mybir as mybir
import concourse.tile as tile
from concourse import bacc
from concourse.bass_utils import run_bass_kernel_spmd

B, S, HID, LAB = 8, 1024, 1024, 64
HD = 68
SCALE = 1.0 / HD**0.5
NCORES = 8
KPAD = 1152  # 9 * 128 contraction rows (1088 channels + 1 bias row + pad)
MTOT = 544   # total projection output channels

F8 = ml_dtypes.float8_e4m3
BF16 = ml_dtypes.bfloat16

# original channel offsets: ent q0 k0 q1 k1 then head q k, tail q k
_ENT_ORIG = [0, 68, 136, 204]
_HT_ORIG = [272, 340, 408, 476]

# ent-group spill pieces: (src_tile, src_row0, cnt, ent_group, dst_row0)
_SPILL = [
    (0, 68, 60, 0, 0), (1, 68, 8, 0, 60),
    (1, 76, 52, 1, 0), (2, 68, 16, 1, 52),
    (2, 84, 44, 2, 0), (3, 68, 24, 2, 44),
    (3, 92, 36, 3, 0), (4, 0, 32, 3, 36),
]

# ent (tril) head rounds, diagonal blocks excluded (host computes those):
# each round is a list of (m_block, local_col0, k_col0, width)
_ENT_ROUNDS = [
    [(0, 0, 128, 896), (6, 896, 896, 128)],
    [(1, 0, 256, 768), (5, 768, 768, 256)],
    [(2, 0, 384, 640), (4, 640, 640, 384)],
    [(3, 0, 512, 512)],
]
_ENT_SPANS = [sum(w for (_, _, _, w) in r) for r in _ENT_ROUNDS]
NROUNDS = 8 + 8 + 4 + 4  # head, tail, ent0, ent1 accumulation columns


def _build_perm():
    """perm[c_new] = original channel index for the projection output layout."""
    perm = np.zeros(MTOT, np.int64)
    for t in range(4):  # head/tail groups aligned at rows 0..67 of tiles 0..3
        perm[t * 128: t * 128 + HD] = np.arange(_HT_ORIG[t], _HT_ORIG[t] + HD)
    for (t, r0, cnt, g, d0) in _SPILL:
        c0 = t * 128 + r0 if t < 4 else 512 + r0
        perm[c0: c0 + cnt] = np.arange(_ENT_ORIG[g] + d0, _ENT_ORIG[g] + d0 + cnt)
    return perm


def _chunks(pieces):
    """Split round pieces into psum-bank-respecting (<=512-col-aligned) matmul
    chunks: (m, local0, kcol0, n)."""
    out = []
    for (m, lo, k0, w) in pieces:
        off = 0
        while off < w:
            n = min(512 - ((lo + off) % 512), w - off)
            out.append((m, lo + off, k0 + off, n))
            off += n
    return out


def _build_nc():
    f32 = mybir.dt.float32
    bf16 = mybir.dt.bfloat16
    fp8 = mybir.dt.float8e4
    Exp = mybir.ActivationFunctionType.Exp
    Mult = mybir.AluOpType.mult
    Add = mybir.AluOpType.add

    nc = bacc.Bacc("TRN2", target_bir_lowering=False)

    xT = nc.dram_tensor("xT", [KPAD, S], fp8, kind="ExternalInput")
    wtb = nc.dram_tensor("wtb", [KPAD, MTOT], fp8, kind="ExternalInput")
    trig = nc.dram_tensor("trig", [HD, 2 * S], bf16, kind="ExternalInput")
    jmat = nc.dram_tensor("jmat", [128, 128], bf16, kind="ExternalInput")
    sums = nc.dram_tensor("sums", [128, NROUNDS], f32, kind="ExternalOutput")
    qkht = nc.dram_tensor("qkht", [HD, 4, S], bf16, kind="ExternalOutput")
    qkent = nc.dram_tensor("qkent", [HD, 4, S], bf16, kind="ExternalOutput")

    xT_r = xT.rearrange("(o p) f -> p o f", p=128)    # [128, 9, 1024]
    wtb_r = wtb.rearrange("(o p) f -> p o f", p=128)  # [128, 9, 544]

    with tile.TileContext(nc) as tc, ExitStack() as ctx:
        singles = ctx.enter_context(tc.tile_pool(name="singles", bufs=1))
        scratch = ctx.enter_context(tc.tile_pool(name="scratch", bufs=2))

        xT_sb = singles.tile([128, 9, S], fp8, tag="xT_sb", name="xT_sb")
        wtb_sb = singles.tile([128, 9, MTOT], fp8, tag="wtb_sb", name="wtb_sb")
        trig_sb = singles.tile([HD, 2 * S], bf16, tag="trig_sb", name="trig_sb")
        jmat_sb = singles.tile([128, 128], bf16, tag="jmat_sb", name="jmat_sb")
        dense = singles.tile([128, 5, S], bf16, tag="dense", name="dense")
        pre = singles.tile([HD, 4, S], bf16, tag="pre", name="pre")
        rot = singles.tile([HD, 4, S], bf16, tag="rot", name="rot")
        sums_sb = singles.tile([128, NROUNDS], f32, tag="sums_sb", name="sums_sb")
        dummy = singles.tile([1, 8], f32, tag="dummy", name="dummy")

        cos_sb = trig_sb[:, 0:S]
        sin_sb = trig_sb[:, S:2 * S]

        # zero accumulators; pre-warm the ACT exp table load
        nc.vector.memset(sums_sb[:], 0.0)
        nc.vector.memset(dummy[:], 0.0)
        nc.scalar.activation(dummy[:], dummy[:], Exp)

        # ---- input DMA issues. sync(SP): xT chunks; gpsimd(SWDGE): wtb,
        # trig, jmat (keeps the ACT engine free for the exp stream). ----
        nc.sync.dma_start(out=xT_sb[:, 0:2], in_=xT_r[:, 0:2])
        nc.gpsimd.dma_start(out=wtb_sb[:, 0:4], in_=wtb_r[:, 0:4])
        nc.sync.dma_start(out=xT_sb[:, 2:4], in_=xT_r[:, 2:4])
        nc.gpsimd.dma_start(out=wtb_sb[:, 4:9], in_=wtb_r[:, 4:9])
        nc.sync.dma_start(out=xT_sb[:, 4:6], in_=xT_r[:, 4:6])
        nc.sync.dma_start(out=xT_sb[:, 6:9], in_=xT_r[:, 6:9])
        nc.gpsimd.dma_start(out=trig_sb[:], in_=trig[:, :])
        nc.gpsimd.dma_start(out=jmat_sb[:], in_=jmat[:, :])

        ps = ctx.enter_context(tc.tile_pool(name="ps", bufs=2, space="PSUM"))

        def proj_mm(t, pt, kt_lo, kt_hi):
            lo = t * 128
            hi = min(lo + 128, MTOT)
            for kt in range(kt_lo, kt_hi):
                for c in (0, 512):
                    nc.tensor.matmul(
                        pt[0:hi - lo, c:c + 512],
                        wtb_sb[:, kt, lo:hi],
                        xT_sb[:, kt, c:c + 512],
                        start=(kt == 0), stop=(kt == 8),
                    )

        def evac(t, pt, eng):
            hi = min(128, MTOT - t * 128)
            if eng == "act":
                nc.scalar.copy(out=dense[0:hi, t, :], in_=pt[0:hi, :])
            else:
                nc.vector.tensor_copy(out=dense[0:hi, t, :], in_=pt[0:hi, :])

        acc = [0]

        def exp_round(pl, qap, kap, pieces):
            span = 0
            for (m, lo, k0, n) in _chunks(pieces):
                nc.tensor.matmul(
                    pl[:, lo:lo + n],
                    qap[:, m * 128:(m + 1) * 128],
                    kap[:, k0:k0 + n],
                    start=True, stop=True,
                )
                span = max(span, lo + n)
            nc.scalar.activation(
                pl[:, 0:span], pl[:, 0:span], Exp, scale=SCALE,
                accum_out=sums_sb[:, acc[0]:acc[0] + 1])
            acc[0] += 1

        def jrot(g):
            pj = ps.tile([128, S], f32, tag="proj", name=f"jq{g}")
            for c in (0, 512):
                nc.tensor.matmul(pj[0:HD, c:c + 512], jmat_sb[0:HD, 0:HD],
                                 pre[:, g, c:c + 512], start=True, stop=True)
            return pj

        def rope(g, pj):
            # rot[g] = pre[g]*cos + (J @ pre[g])*sin
            rtmp = scratch.tile([HD, S], bf16, tag="rtmp", name=f"rtmp{g}")
            nc.vector.tensor_tensor(rtmp[:, :], pj[0:HD, :], sin_sb, Mult)
            nc.vector.tensor_tensor(rot[:, g, :], pre[:, g, :], cos_sb, Mult)
            nc.vector.tensor_tensor(rot[:, g, :], rot[:, g, :], rtmp[:, :], Add)

        # ---- phase 1: proj tiles 0,1 (q_head, k_head), kt-major so the PE
        # rides the incoming xT/wtb stream ----
        pt0 = ps.tile([128, S], f32, tag="proj", name="proj0")
        pt1 = ps.tile([128, S], f32, tag="proj", name="proj1")
        for kt in range(9):
            for t, pt in ((0, pt0), (1, pt1)):
                for c in (0, 512):
                    nc.tensor.matmul(pt[:, c:c + 512],
                                     wtb_sb[:, kt, t * 128:(t + 1) * 128],
                                     xT_sb[:, kt, c:c + 512],
                                     start=(kt == 0), stop=(kt == 8))
        evac(0, pt0, "dve")
        evac(1, pt1, "act")

        q_head, k_head = dense[0:HD, 0, :], dense[0:HD, 1, :]
        q_tail, k_tail = dense[0:HD, 2, :], dense[0:HD, 3, :]

        # ---- head stream (8 rounds) with proj phase 2 interleaved on PE ----
        pt2 = pt3 = pt4 = None
        for m in range(8):
            pl = ps.tile([128, S], f32, tag="round", name=f"h_{m}")
            exp_round(pl, q_head, k_head, [(m, 0, 0, 1024)])
            if m == 1:
                pt2 = ps.tile([128, S], f32, tag="proj", name="proj2")
                proj_mm(2, pt2, 0, 5)
            elif m == 2:
                proj_mm(2, pt2, 5, 9)
                evac(2, pt2, "dve")
            elif m == 3:
                pt3 = ps.tile([128, S], f32, tag="proj", name="proj3")
                proj_mm(3, pt3, 0, 5)
            elif m == 4:
                proj_mm(3, pt3, 5, 9)
                evac(3, pt3, "dve")
            elif m == 5:
                pt4 = ps.tile([128, S], f32, tag="proj", name="proj4")
                proj_mm(4, pt4, 0, 9)
                evac(4, pt4, "dve")

        # ---- ent-group assembly spills (sync queue, SBUF->SBUF) ----
        for (t, r0, cnt, g, d0) in _SPILL:
            nc.sync.dma_start(out=pre[d0:d0 + cnt, g, :],
                              in_=dense[r0:r0 + cnt, t, :])

        # head/tail q,k out for the host-side corrections
        nc.gpsimd.dma_start(out=qkht[:, :, :], in_=dense[0:HD, 0:4, :])

        # ---- tail stream (8 rounds) with jrot/rope interleaved ----
        for m in range(8):
            pl = ps.tile([128, S], f32, tag="round", name=f"t_{m}")
            exp_round(pl, q_tail, k_tail, [(m, 0, 0, 1024)])
            if m < 4:
                pj = jrot(m)
                rope(m, pj)

        nc.gpsimd.dma_start(out=qkent[:, :, :], in_=rot[:, :, :])

        # ---- ent heads (4 rounds each, diagonal blocks left to host) ----
        for h in range(2):
            qap, kap = rot[:, 2 * h, :], rot[:, 2 * h + 1, :]
            for ri, pieces in enumerate(_ENT_ROUNDS):
                pl = ps.tile([128, S], f32, tag="round", name=f"e{h}_{ri}")
                exp_round(pl, qap, kap, pieces)

        nc.sync.dma_start(out=sums[:, :], in_=sums_sb[:, :])

    nc.finalize()
    return nc


_NC_CACHE = None


def _get_nc():
    global _NC_CACHE
    if _NC_CACHE is None:
        _NC_CACHE = _build_nc()
    return _NC_CACHE


def _host_tables():
    pos = np.arange(S, dtype=np.float64)[:, None]
    inv = np.power(10000.0, -2.0 * np.arange(HD // 2, dtype=np.float64) / HD)
    ang = pos * inv                                   # [S, 34]
    trig = np.zeros((HD, 2 * S), np.float32)
    trig[:, 0:S] = np.repeat(np.cos(ang), 2, axis=1).T
    trig[:, S:2 * S] = np.repeat(np.sin(ang), 2, axis=1).T
    jmat = np.zeros((128, 128), np.float32)
    for i in range(HD // 2):
        # J[2i, 2i+1] = -1 ; J[2i+1, 2i] = +1  -> stored transposed
        jmat[2 * i + 1, 2 * i] = -1.0
        jmat[2 * i, 2 * i + 1] = 1.0
    return trig.astype(BF16), jmat.astype(BF16)


def _mcce_host(E_dev, q, k, gt):
    """pos/neg multilabel-CE for one (example, head). q,k: [68,S] f64; gt: [P,2]."""
    i = gt[:, 0].astype(np.int64)
    j = gt[:, 1].astype(np.int64)
    flat = i * S + j
    lv = np.sum(q[:, i] * k[:, j], axis=0) * SCALE    # [P]
    live = flat != 0
    pos_loss = np.log1p(np.sum(np.exp(-lv[live])))
    l00 = float(np.sum(q[:, 0] * k[:, 0]) * SCALE)
    uf, ui = np.unique(flat, return_index=True)
    keep = uf != 0
    excl = np.exp(l00) + np.sum(np.exp(lv[ui[keep]]))
    neg_loss = np.log1p(E_dev - excl)
    return pos_loss + neg_loss


_DIAG_IU = np.triu_indices(128)


def _diag_E(q, k):
    """Upper-tri (incl diagonal) exp-sum of the 8 diagonal 128x128 blocks."""
    qb = q.reshape(HD, 8, 128)
    kb = k.reshape(HD, 8, 128)
    blocks = np.einsum('cmi,cmj->mij', qb, kb)        # [8,128,128] f64
    vals = blocks[:, _DIAG_IU[0], _DIAG_IU[1]] * SCALE
    return float(np.sum(np.exp(vals)))


def _reference_numpy(hidden, entity_labels, attention_mask, gt_entity, gt_head,
                     gt_tail, ent_emb, W_ent, b_ent, W_head, b_head, W_tail,
                     b_tail):
    """Slow exact numpy fallback (used only if attention_mask is not all-ones)."""
    INF = 1.0e12
    x = np.concatenate([hidden, ent_emb[entity_labels]], axis=-1)

    def rope_np(v):
        b, s, h, d = v.shape
        pos = np.arange(s, dtype=np.float32)[:, None]
        inv = np.power(10000.0, -2.0 * np.arange(d // 2, dtype=np.float32) / d)
        ang = pos * inv
        sin = np.repeat(np.sin(ang), 2, axis=-1)[None, :, None, :]
        cos = np.repeat(np.cos(ang), 2, axis=-1)[None, :, None, :]
        v2 = np.stack([-v[..., 1::2], v[..., ::2]], axis=-1).reshape(v.shape)
        return v * cos + v2 * sin

    def gp(x, W, b, mask, heads, use_rope, tril):
        bx, sx, _ = x.shape
        proj = (x @ W.T + b).reshape(bx, sx, heads, 2 * HD)
        qw, kw = proj[..., :HD], proj[..., HD:]
        if use_rope:
            qw, kw = rope_np(qw), rope_np(kw)
        logits = np.einsum('bmhd,bnhd->bhmn', qw, kw) * SCALE
        pad = mask[:, None, None, :]
        logits = logits * pad - (1.0 - pad) * INF
        if tril:
            logits = logits - np.tril(np.ones((sx, sx), np.float32), -1) * INF
        return logits

    def mcce(y_true, y_pred):
        bx, hx, sx, _ = y_pred.shape
        flat = y_true[..., 0].astype(np.int64) * sx + y_true[..., 1]
        yp = y_pred.reshape(bx, hx, sx * sx).astype(np.float64)
        total = 0.0
        for b in range(bx):
            for h in range(hx):
                f = flat[b, h]
                live = f != 0
                lv = yp[b, h][f]
                pos = np.log1p(np.sum(np.exp(-lv[live])))
                neg_terms = yp[b, h].copy()
                neg_terms[0] = -np.inf
                neg_terms[np.unique(f)] = -np.inf
                neg = np.log1p(np.sum(np.exp(neg_terms)))
                total += pos + neg
        return total

    loss = 0.0
    loss += mcce(gt_entity, gp(x, W_ent, b_ent, attention_mask, 2, True, True))
    loss += mcce(gt_head, gp(x, W_head, b_head, attention_mask, 1, False, False))
    loss += mcce(gt_tail, gp(x, W_tail, b_tail, attention_mask, 1, False, False))
    return np.array(loss, dtype=np.float32)


def kernel(hidden, entity_labels, attention_mask, gt_entity, gt_head, gt_tail,
           ent_emb, W_ent, b_ent, W_head, b_head, W_tail, b_tail,
           _want_trace=False):
    hidden = np.asarray(hidden, np.float32)
    entity_labels = np.asarray(entity_labels)
    attention_mask = np.asarray(attention_mask, np.float32)
    ent_emb = np.asarray(ent_emb, np.float32)

    if not np.all(attention_mask == 1.0):
        return _reference_numpy(
            hidden, entity_labels, attention_mask, np.asarray(gt_entity),
            np.asarray(gt_head), np.asarray(gt_tail), ent_emb,
            np.asarray(W_ent, np.float32), np.asarray(b_ent, np.float32),
            np.asarray(W_head, np.float32), np.asarray(b_head, np.float32),
            np.asarray(W_tail, np.float32), np.asarray(b_tail, np.float32))

    W_all = np.concatenate(
        [np.asarray(W_ent, np.float32), np.asarray(W_head, np.float32),
         np.asarray(W_tail, np.float32)], axis=0)       # [544, 1088]
    b_all = np.concatenate(
        [np.asarray(b_ent, np.float32), np.asarray(b_head, np.float32),
         np.asarray(b_tail, np.float32)], axis=0)       # [544]
    perm = _build_perm()
    Wp, bp = W_all[perm], b_all[perm]
    wtb = np.zeros((KPAD, MTOT), np.float32)
    wtb[:HID + LAB] = Wp.T
    wtb[HID + LAB] = bp
    wtb = wtb.astype(F8)

    trig, jmat = _host_tables()

    in_maps = []
    for b in range(B):
        xT = np.zeros((KPAD, S), np.float32)
        xT[:HID] = hidden[b].T
        xT[HID:HID + LAB] = ent_emb[entity_labels[b]].T
        xT[HID + LAB] = 1.0
        in_maps.append(dict(xT=xT.astype(F8), wtb=wtb, trig=trig, jmat=jmat))

    nc = _get_nc()
    res = run_bass_kernel_spmd(nc, in_maps, core_ids=list(range(NCORES)),
                               trace=_want_trace)

    gt_entity = np.asarray(gt_entity)
    gt_head = np.asarray(gt_head)
    gt_tail = np.asarray(gt_tail)
    total = 0.0
    for b in range(B):
        out = res.results[b]
        sums = np.asarray(out["sums"]).astype(np.float64)  # [128, NROUNDS]
        ht = np.asarray(out["qkht"]).astype(np.float64)    # [68, 4, S]
        en = np.asarray(out["qkent"]).astype(np.float64)   # [68, 4, S]
        col = np.sum(sums, axis=0)                         # [NROUNDS]
        # head (cols 0:8), tail (8:16), ent0 (16:20), ent1 (20:24)
        total += _mcce_host(np.sum(col[0:8]), ht[:, 0], ht[:, 1], gt_head[b, 0])
        total += _mcce_host(np.sum(col[8:16]), ht[:, 2], ht[:, 3], gt_tail[b, 0])
        for h in range(2):
            q, k = en[:, 2 * h], en[:, 2 * h + 1]
            E = np.sum(col[16 + 4 * h:20 + 4 * h]) + _diag_E(q, k)
            total += _mcce_host(E, q, k, gt_entity[b, h])

    if _want_trace:
        kernel._last_results = res
    return np.array(total, dtype=np.float32)


# revision 11
# speedup vs baseline: 1.0689x; 1.0689x over previous
"""Bass/Trainium2 kernel for nn_GPREDecoder (GlobalPointer relation-extraction loss).

Strategy: data-parallel over batch (B=8 -> 8 cores, 1 example per core).
Per example the device computes:
  - projT = W_all @ x_aug.T in fp8(e4m3) inputs -> psum f32, streamed so the
    PE consumes input kt-chunks as the DMA delivers them
  - the projection output channel order is PERMUTED so the four non-rope
    groups (q_head, k_head, q_tail, k_tail) land row-aligned (rows 0..67) in
    proj tiles 0..3 -> the head/tail exp streams start right after evac
  - the four rope (ent) groups are spilled/assembled in the background, then
    RoPE-rotated (J-matmul + cos/sin elementwise in bf16)
  - per-head S x S logit tiles on PE (bf16), exp(SCALE*logit) on ACT with
    fused per-row accumulation -> per-head sum(exp(logits))
  - tril heads: sub-diagonal blocks are simply never computed; the DIAGONAL
    128x128 blocks are left off-device entirely (host adds their upper-tri
    exp-sum from the returned q/k), so no masks exist on the device
The host gathers the 64 ground-truth pairs per head from q/k, computes the
diagonal-block corrections for the tril heads, applies the multilabel-CE
pos/neg log corrections in float64, and returns the scalar loss.
"""

import ml_dtypes
import numpy as np
from contextlib import ExitStack

import concourse.bass as bass
import concourse.mybir as mybir
import concourse.tile as tile
from concourse import bacc
from concourse.bass_utils import run_bass_kernel_spmd

B, S, HID, LAB = 8, 1024, 1024, 64
HD = 68
SCALE = 1.0 / HD**0.5
NCORES = 8
KPAD = 1152  # 9 * 128 contraction rows (1088 channels + 1 bias row + pad)
MTOT = 544   # total projection output channels

F8 = ml_dtypes.float8_e4m3
BF16 = ml_dtypes.bfloat16

# original channel offsets: ent q0 k0 q1 k1 then head q k, tail q k
_ENT_ORIG = [0, 68, 136, 204]
_HT_ORIG = [272, 340, 408, 476]

# ent-group spill pieces: (src_tile, src_row0, cnt, ent_group, dst_row0)
_SPILL = [
    (0, 68, 60, 0, 0), (1, 68, 8, 0, 60),
    (1, 76, 52, 1, 0), (2, 68, 16, 1, 52),
    (2, 84, 44, 2, 0), (3, 68, 24, 2, 44),
    (3, 92, 36, 3, 0), (4, 0, 32, 3, 36),
]

# ent (tril) head rounds, diagonal blocks excluded (host computes those):
# each round is a list of (m_block, local_col0, k_col0, width)
_ENT_ROUNDS = [
    [(0, 0, 128, 896), (6, 896, 896, 128)],
    [(1, 0, 256, 768), (5, 768, 768, 256)],
    [(2, 0, 384, 640), (4, 640, 640, 384)],
    [(3, 0, 512, 512)],
]
_ENT_SPANS = [sum(w for (_, _, _, w) in r) for r in _ENT_ROUNDS]
NROUNDS = 8 + 8 + 4 + 4  # head, tail, ent0, ent1 accumulation columns


def _build_perm():
    """perm[c_new] = original channel index for the projection output layout."""
    perm = np.zeros(MTOT, np.int64)
    for t in range(4):  # head/tail groups aligned at rows 0..67 of tiles 0..3
        perm[t * 128: t * 128 + HD] = np.arange(_HT_ORIG[t], _HT_ORIG[t] + HD)
    for (t, r0, cnt, g, d0) in _SPILL:
        c0 = t * 128 + r0 if t < 4 else 512 + r0
        perm[c0: c0 + cnt] = np.arange(_ENT_ORIG[g] + d0, _ENT_ORIG[g] + d0 + cnt)
    return perm


def _chunks(pieces):
    """Split round pieces into psum-bank-respecting (<=512-col-aligned) matmul
    chunks: (m, local0, kcol0, n)."""
    out = []
    for (m, lo, k0, w) in pieces:
        off = 0
        while off < w:
            n = min(512 - ((lo + off) % 512), w - off)
            out.append((m, lo + off, k0 + off, n))
            off += n
    return out


def _build_nc():
    f32 = mybir.dt.float32
    bf16 = mybir.dt.bfloat16
    fp8 = mybir.dt.float8e4
    Exp = mybir.ActivationFunctionType.Exp
    Mult = mybir.AluOpType.mult
    Add = mybir.AluOpType.add
    DR = mybir.MatmulPerfMode.DoubleRow

    nc = bacc.Bacc("TRN2", target_bir_lowering=False)

    xT = nc.dram_tensor("xT", [KPAD, S], fp8, kind="ExternalInput")
    wtb = nc.dram_tensor("wtb", [KPAD, MTOT], fp8, kind="ExternalInput")
    trig = nc.dram_tensor("trig", [HD, 2 * S], bf16, kind="ExternalInput")
    jmat = nc.dram_tensor("jmat", [128, 128], bf16, kind="ExternalInput")
    sums = nc.dram_tensor("sums", [128, NROUNDS], f32, kind="ExternalOutput")
    qkht = nc.dram_tensor("qkht", [HD, 4, S], bf16, kind="ExternalOutput")
    qkent = nc.dram_tensor("qkent", [HD, 4, S], bf16, kind="ExternalOutput")

    xT_r = xT.rearrange("(o p) f -> p o f", p=128)    # [128, 9, 1024]
    wtb_r = wtb.rearrange("(o p) f -> p o f", p=128)  # [128, 9, 544]

    with tile.TileContext(nc) as tc, ExitStack() as ctx:
        singles = ctx.enter_context(tc.tile_pool(name="singles", bufs=1))
        scratch = ctx.enter_context(tc.tile_pool(name="scratch", bufs=2))

        xT_sb = singles.tile([128, 9, S], fp8, tag="xT_sb", name="xT_sb")
        wtb_sb = singles.tile([128, 9, MTOT], fp8, tag="wtb_sb", name="wtb_sb")
        trig_sb = singles.tile([HD, 2 * S], bf16, tag="trig_sb", name="trig_sb")
        jmat_sb = singles.tile([128, 128], bf16, tag="jmat_sb", name="jmat_sb")
        warm_sb = singles.tile([128, 512], bf16, tag="warm_sb", name="warm_sb")
        dense = singles.tile([128, 5, S], bf16, tag="dense", name="dense")
        pre = singles.tile([HD, 4, S], bf16, tag="pre", name="pre")
        rot = singles.tile([HD, 4, S], bf16, tag="rot", name="rot")
        sums_sb = singles.tile([128, NROUNDS], f32, tag="sums_sb", name="sums_sb")
        dummy = singles.tile([1, 8], f32, tag="dummy", name="dummy")

        cos_sb = trig_sb[:, 0:S]
        sin_sb = trig_sb[:, S:2 * S]

        # zero accumulators; pre-warm the ACT exp table load
        nc.vector.memset(sums_sb[:], 0.0)
        nc.vector.memset(dummy[:], 0.0)
        nc.vector.memset(warm_sb[:], 0.0)
        nc.scalar.activation(dummy[:], dummy[:], Exp)

        # ---- input DMA issues. sync(SP): xT chunks; gpsimd(SWDGE): wtb,
        # trig, jmat (keeps the ACT engine free for the exp stream). ----
        nc.sync.dma_start(out=xT_sb[:, 0:2], in_=xT_r[:, 0:2])
        nc.gpsimd.dma_start(out=wtb_sb[:, 0:4], in_=wtb_r[:, 0:4])
        nc.sync.dma_start(out=xT_sb[:, 2:4], in_=xT_r[:, 2:4])
        nc.gpsimd.dma_start(out=wtb_sb[:, 4:9], in_=wtb_r[:, 4:9])
        nc.sync.dma_start(out=xT_sb[:, 4:6], in_=xT_r[:, 4:6])
        nc.sync.dma_start(out=xT_sb[:, 6:9], in_=xT_r[:, 6:9])
        nc.gpsimd.dma_start(out=trig_sb[:], in_=trig[:, :])
        nc.gpsimd.dma_start(out=jmat_sb[:], in_=jmat[:, :])

        ps = ctx.enter_context(tc.tile_pool(name="ps", bufs=2, space="PSUM"))

        def proj_mm(t, pt, p_lo, p_hi):
            """kt-pair DoubleRow matmuls [p_lo, p_hi) for tile t; pair 4 is
            the single trailing kt=8 in normal fp8 mode."""
            lo = t * 128
            hi = min(lo + 128, MTOT)
            for p in range(p_lo, p_hi):
                for c in (0, 512):
                    if p < 4:
                        nc.tensor.matmul(
                            pt[0:hi - lo, c:c + 512],
                            wtb_sb[:, 2 * p:2 * p + 2, lo:hi],
                            xT_sb[:, 2 * p:2 * p + 2, c:c + 512],
                            start=(p == 0), stop=False, perf_mode=DR,
                        )
                    else:
                        nc.tensor.matmul(
                            pt[0:hi - lo, c:c + 512],
                            wtb_sb[:, 8, lo:hi],
                            xT_sb[:, 8, c:c + 512],
                            start=False, stop=True,
                        )

        def evac(t, pt, eng):
            hi = min(128, MTOT - t * 128)
            if eng == "act":
                nc.scalar.copy(out=dense[0:hi, t, :], in_=pt[0:hi, :])
            else:
                nc.vector.tensor_copy(out=dense[0:hi, t, :], in_=pt[0:hi, :])

        acc = [0]

        def exp_round(pl, qap, kap, pieces):
            span = 0
            for (m, lo, k0, n) in _chunks(pieces):
                nc.tensor.matmul(
                    pl[:, lo:lo + n],
                    qap[:, m * 128:(m + 1) * 128],
                    kap[:, k0:k0 + n],
                    start=True, stop=True,
                )
                span = max(span, lo + n)
            nc.scalar.activation(
                pl[:, 0:span], pl[:, 0:span], Exp, scale=SCALE,
                accum_out=sums_sb[:, acc[0]:acc[0] + 1])
            acc[0] += 1

        def jrot(g):
            pj = ps.tile([128, S], f32, tag="proj", name=f"jq{g}")
            for c in (0, 512):
                nc.tensor.matmul(pj[0:HD, c:c + 512], jmat_sb[0:HD, 0:HD],
                                 pre[:, g, c:c + 512], start=True, stop=True)
            return pj

        def rope(g, pj):
            # rot[g] = pre[g]*cos + (J @ pre[g])*sin
            rtmp = scratch.tile([HD, S], bf16, tag="rtmp", name=f"rtmp{g}")
            nc.vector.tensor_tensor(rtmp[:, :], pj[0:HD, :], sin_sb, Mult)
            nc.vector.tensor_tensor(rot[:, g, :], pre[:, g, :], cos_sb, Mult)
            nc.vector.tensor_tensor(rot[:, g, :], rot[:, g, :], rtmp[:, :], Add)

        # ---- PE warmup: no-dep matmuls on a zeroed tile keep the Tensor
        # engine continuously busy (and its p-state ramping toward 2.4 GHz)
        # while the first input chunks stream in ----
        wm = ps.tile([128, 512], f32, tag="round", name="warm_ps")
        for _ in range(14):
            nc.tensor.matmul(wm[:, 0:512], warm_sb[:, 0:128],
                             warm_sb[:, 0:512], start=True, stop=True)

        # ---- phase 1: proj tiles 0,1 (q_head, k_head), kt-pair-major so the
        # PE rides the incoming xT/wtb stream ----
        pt0 = ps.tile([128, S], f32, tag="proj", name="proj0")
        pt1 = ps.tile([128, S], f32, tag="proj", name="proj1")
        for p in range(5):
            proj_mm(0, pt0, p, p + 1)
            proj_mm(1, pt1, p, p + 1)
        evac(0, pt0, "dve")
        evac(1, pt1, "act")

        q_head, k_head = dense[0:HD, 0, :], dense[0:HD, 1, :]
        q_tail, k_tail = dense[0:HD, 2, :], dense[0:HD, 3, :]

        # ---- head stream (8 rounds) with proj phase 2 interleaved on PE ----
        pt2 = pt3 = pt4 = None
        for m in range(8):
            pl = ps.tile([128, S], f32, tag="round", name=f"h_{m}")
            exp_round(pl, q_head, k_head, [(m, 0, 0, 1024)])
            if m == 1:
                pt2 = ps.tile([128, S], f32, tag="proj", name="proj2")
                proj_mm(2, pt2, 0, 3)
            elif m == 2:
                proj_mm(2, pt2, 3, 5)
                evac(2, pt2, "dve")
            elif m == 3:
                pt3 = ps.tile([128, S], f32, tag="proj", name="proj3")
                proj_mm(3, pt3, 0, 3)
            elif m == 4:
                proj_mm(3, pt3, 3, 5)
                evac(3, pt3, "dve")
            elif m == 5:
                pt4 = ps.tile([128, S], f32, tag="proj", name="proj4")
                proj_mm(4, pt4, 0, 5)
                evac(4, pt4, "dve")

        # ---- ent-group assembly spills (sync queue, SBUF->SBUF) ----
        for (t, r0, cnt, g, d0) in _SPILL:
            nc.sync.dma_start(out=pre[d0:d0 + cnt, g, :],
                              in_=dense[r0:r0 + cnt, t, :])
        nc.sync.dma_start(out=sums[:, 0:8], in_=sums_sb[:, 0:8])

        # head/tail q,k out for the host-side corrections
        nc.gpsimd.dma_start(out=qkht[:, :, :], in_=dense[0:HD, 0:4, :])

        # ---- tail stream (8 rounds) with jrot/rope interleaved ----
        for m in range(8):
            pl = ps.tile([128, S], f32, tag="round", name=f"t_{m}")
            exp_round(pl, q_tail, k_tail, [(m, 0, 0, 1024)])
            if m < 4:
                pj = jrot(m)
                rope(m, pj)

        nc.gpsimd.dma_start(out=qkent[:, :, :], in_=rot[:, :, :])
        nc.sync.dma_start(out=sums[:, 8:16], in_=sums_sb[:, 8:16])

        # ---- ent heads (4 rounds each, diagonal blocks left to host) ----
        for h in range(2):
            qap, kap = rot[:, 2 * h, :], rot[:, 2 * h + 1, :]
            for ri, pieces in enumerate(_ENT_ROUNDS):
                pl = ps.tile([128, S], f32, tag="round", name=f"e{h}_{ri}")
                exp_round(pl, qap, kap, pieces)

        nc.sync.dma_start(out=sums[:, 16:24], in_=sums_sb[:, 16:24])

    nc.finalize()
    return nc


_NC_CACHE = None


def _get_nc():
    global _NC_CACHE
    if _NC_CACHE is None:
        _NC_CACHE = _build_nc()
    return _NC_CACHE


def _host_tables():
    pos = np.arange(S, dtype=np.float64)[:, None]
    inv = np.power(10000.0, -2.0 * np.arange(HD // 2, dtype=np.float64) / HD)
    ang = pos * inv                                   # [S, 34]
    trig = np.zeros((HD, 2 * S), np.float32)
    trig[:, 0:S] = np.repeat(np.cos(ang), 2, axis=1).T
    trig[:, S:2 * S] = np.repeat(np.sin(ang), 2, axis=1).T
    jmat = np.zeros((128, 128), np.float32)
    for i in range(HD // 2):
        # J[2i, 2i+1] = -1 ; J[2i+1, 2i] = +1  -> stored transposed
        jmat[2 * i + 1, 2 * i] = -1.0
        jmat[2 * i, 2 * i + 1] = 1.0
    return trig.astype(BF16), jmat.astype(BF16)


def _mcce_host(E_dev, q, k, gt):
    """pos/neg multilabel-CE for one (example, head). q,k: [68,S] f64; gt: [P,2]."""
    i = gt[:, 0].astype(np.int64)
    j = gt[:, 1].astype(np.int64)
    flat = i * S + j
    lv = np.sum(q[:, i] * k[:, j], axis=0) * SCALE    # [P]
    live = flat != 0
    pos_loss = np.log1p(np.sum(np.exp(-lv[live])))
    l00 = float(np.sum(q[:, 0] * k[:, 0]) * SCALE)
    uf, ui = np.unique(flat, return_index=True)
    keep = uf != 0
    excl = np.exp(l00) + np.sum(np.exp(lv[ui[keep]]))
    neg_loss = np.log1p(E_dev - excl)
    return pos_loss + neg_loss


_DIAG_IU = np.triu_indices(128)


def _diag_E(q, k):
    """Upper-tri (incl diagonal) exp-sum of the 8 diagonal 128x128 blocks."""
    qb = q.reshape(HD, 8, 128)
    kb = k.reshape(HD, 8, 128)
    blocks = np.einsum('cmi,cmj->mij', qb, kb)        # [8,128,128] f64
    vals = blocks[:, _DIAG_IU[0], _DIAG_IU[1]] * SCALE
    return float(np.sum(np.exp(vals)))


def _reference_numpy(hidden, entity_labels, attention_mask, gt_entity, gt_head,
                     gt_tail, ent_emb, W_ent, b_ent, W_head, b_head, W_tail,
                     b_tail):
    """Slow exact numpy fallback (used only if attention_mask is not all-ones)."""
    INF = 1.0e12
    x = np.concatenate([hidden, ent_emb[entity_labels]], axis=-1)

    def rope_np(v):
        b, s, h, d = v.shape
        pos = np.arange(s, dtype=np.float32)[:, None]
        inv = np.power(10000.0, -2.0 * np.arange(d // 2, dtype=np.float32) / d)
        ang = pos * inv
        sin = np.repeat(np.sin(ang), 2, axis=-1)[None, :, None, :]
        cos = np.repeat(np.cos(ang), 2, axis=-1)[None, :, None, :]
        v2 = np.stack([-v[..., 1::2], v[..., ::2]], axis=-1).reshape(v.shape)
        return v * cos + v2 * sin

    def gp(x, W, b, mask, heads, use_rope, tril):
        bx, sx, _ = x.shape
        proj = (x @ W.T + b).reshape(bx, sx, heads, 2 * HD)
        qw, kw = proj[..., :HD], proj[..., HD:]
        if use_rope:
            qw, kw = rope_np(qw), rope_np(kw)
        logits = np.einsum('bmhd,bnhd->bhmn', qw, kw) * SCALE
        pad = mask[:, None, None, :]
        logits = logits * pad - (1.0 - pad) * INF
        if tril:
            logits = logits - np.tril(np.ones((sx, sx), np.float32), -1) * INF
        return logits

    def mcce(y_true, y_pred):
        bx, hx, sx, _ = y_pred.shape
        flat = y_true[..., 0].astype(np.int64) * sx + y_true[..., 1]
        yp = y_pred.reshape(bx, hx, sx * sx).astype(np.float64)
        total = 0.0
        for b in range(bx):
            for h in range(hx):
                f = flat[b, h]
                live = f != 0
                lv = yp[b, h][f]
                pos = np.log1p(np.sum(np.exp(-lv[live])))
                neg_terms = yp[b, h].copy()
                neg_terms[0] = -np.inf
                neg_terms[np.unique(f)] = -np.inf
                neg = np.log1p(np.sum(np.exp(neg_terms)))
                total += pos + neg
        return total

    loss = 0.0
    loss += mcce(gt_entity, gp(x, W_ent, b_ent, attention_mask, 2, True, True))
    loss += mcce(gt_head, gp(x, W_head, b_head, attention_mask, 1, False, False))
    loss += mcce(gt_tail, gp(x, W_tail, b_tail, attention_mask, 1, False, False))
    return np.array(loss, dtype=np.float32)


def kernel(hidden, entity_labels, attention_mask, gt_entity, gt_head, gt_tail,
           ent_emb, W_ent, b_ent, W_head, b_head, W_tail, b_tail,
           _want_trace=False):
    hidden = np.asarray(hidden, np.float32)
    entity_labels = np.asarray(entity_labels)
    attention_mask = np.asarray(attention_mask, np.float32)
    ent_emb = np.asarray(ent_emb, np.float32)

    if not np.all(attention_mask == 1.0):
        return _reference_numpy(
            hidden, entity_labels, attention_mask, np.asarray(gt_entity),
            np.asarray(gt_head), np.asarray(gt_tail), ent_emb,
            np.asarray(W_ent, np.float32), np.asarray(b_ent, np.float32),
            np.asarray(W_head, np.float32), np.asarray(b_head, np.float32),
            np.asarray(W_tail, np.float32), np.asarray(b_tail, np.float32))

    W_all = np.concatenate(
        [np.asarray(W_ent, np.float32), np.asarray(W_head, np.float32),
         np.asarray(W_tail, np.float32)], axis=0)       # [544, 1088]
    b_all = np.concatenate(
        [np.asarray(b_ent, np.float32), np.asarray(b_head, np.float32),
         np.asarray(b_tail, np.float32)], axis=0)       # [544]
    perm = _build_perm()
    Wp, bp = W_all[perm], b_all[perm]
    wtb = np.zeros((KPAD, MTOT), np.float32)
    wtb[:HID + LAB] = Wp.T
    wtb[HID + LAB] = bp
    wtb = wtb.astype(F8)

    trig, jmat = _host_tables()

    in_maps = []
    for b in range(B):
        xT = np.zeros((KPAD, S), np.float32)
        xT[:HID] = hidden[b].T
        xT[HID:HID + LAB] = ent_emb[entity_labels[b]].T
        xT[HID + LAB] = 1.0
        in_maps.append(dict(xT=xT.astype(F8), wtb=wtb, trig=trig, jmat=jmat))

    nc = _get_nc()
    res = run_bass_kernel_spmd(nc, in_maps, core_ids=list(range(NCORES)),
                               trace=_want_trace)

    gt_entity = np.asarray(gt_entity)
    gt_head = np.asarray(gt_head)
    gt_tail = np.asarray(gt_tail)
    total = 0.0
    for b in range(B):
        out = res.results[b]
        sums = np.asarray(out["sums"]).astype(np.float64)  # [128, NROUNDS]
        ht = np.asarray(out["qkht"]).astype(np.float64)    # [68, 4, S]
        en = np.asarray(out["qkent"]).astype(np.float64)   # [68, 4, S]
        col = np.sum(sums, axis=0)                         # [NROUNDS]
        # head (cols 0:8), tail (8:16), ent0 (16:20), ent1 (20:24)
        total += _mcce_host(np.sum(col[0:8]), ht[:, 0], ht[:, 1], gt_head[b, 0])
        total += _mcce_host(np.sum(col[8:16]), ht[:, 2], ht[:, 3], gt_tail[b, 0])
        for h in range(2):
            q, k = en[:, 2 * h], en[:, 2 * h + 1]
            E = np.sum(col[16 + 4 * h:20 + 4 * h]) + _diag_E(q, k)
            total += _mcce_host(E, q, k, gt_entity[b, h])

    if _want_trace:
        kernel._last_results = res
    return np.array(total, dtype=np.float32)
